# revision 8
# baseline (speedup 1.0000x reference)
"""Trainium2 Bass kernel for nn_BasicBlockBit (ResNet BasicBlock, ternary convs).

Math (per reference):
    out = silu(bn2(conv3x3(silu(bn1(conv3x3(x, q(w1)) + b1)), q(w2)) + b2) + x)
with q() = BitNet ternary quantization (per-tensor median scale).

Strategy:
  - Pure data parallelism: batch 32 -> 4 images per core across 8 cores.
  - Mixed-precision taps: per 3x3 conv, a few taps run in fp16 (exact) and the
    rest run as fp8-e4m3 DoubleRow matmuls (2 taps per PE instruction at 2x
    ALU rate). conv1: 3 exact taps (middle row) + 3 DR pairs; conv2: 1 exact
    tap (center) + 4 DR pairs. Measured end-to-end rel err ~1.88e-2 < 2e-2.
  - Separator layout: image rows are stored with stride 113 (112 pixels + one
    zero column) plus a zero pad row above/below. Every 3x3 tap window of a
    4-row block is then one contiguous 452-element slab whose out-of-image
    reads land on zeros == exact zero padding. DR pairs use a custom
    overlapping AP [128, 2(step=tap delta), 452].
  - Epilogues: conv1: ACT Silu(ps*scale1+bias1) -> fp16 mid, DVE copy to fp8
    mid. conv2: DVE affine, DVE residual add (fp16 x), ACT Silu -> fp16
    staging -> DMA out (host upcasts to f32).
"""

import sys

import numpy as np
import ml_dtypes

try:  # concourse normally resolves via the environment's sitecustomize
    import concourse  # noqa: F401
except ImportError:  # pragma: no cover
    sys.path.insert(0, "/opt/trn_rl_repo")

C = 128
H = W = 112
SW = 113                 # row stride: 112 pixels + 1 zero separator
LROWS = 114              # pad row + 112 rows + pad row
L = SW * LROWS + 14      # 12896; window reads reach index 12883
NPC = 4                  # images per core
NCORES = 8
RB = 4                   # image rows per PSUM tile
NPIX = RB * SW           # 452 psum columns (448 valid)
NOUT = RB * W            # 448
NBLK = H // RB           # 28
BN_EPS = 1e-5

# taps as (dy, dx) in {-1,0,1}; offset in separator layout = 113*dy + dx
def _toff(dy, dx):
    return SW * dy + dx

# conv1: exact (fp16) taps = middle row; fp8 DR pairs (a first = smaller off)
C1_EXACT = [(0, -1), (0, 0), (0, 1)]
C1_PAIRS = [((-1, -1), (-1, 1)), ((-1, 0), (1, 0)), ((1, -1), (1, 1))]
# conv2: exact tap = center; 4 DR pairs
C2_EXACT = [(0, 0)]
C2_PAIRS = [((-1, -1), (-1, 1)), ((0, -1), (0, 1)), ((1, -1), (1, 1)),
            ((-1, 0), (1, 0))]

_CACHE = {}


def _build_nc():
    import concourse.mybir as mybir
    from concourse import bacc, bass
    from concourse.tile import TileContext

    f32 = mybir.dt.float32
    f16 = mybir.dt.float16
    f8 = mybir.dt.float8e4
    bf16 = mybir.dt.bfloat16
    DR = mybir.MatmulPerfMode.DoubleRow
    Silu = mybir.ActivationFunctionType.Silu
    mult = mybir.AluOpType.mult
    add = mybir.AluOpType.add

    nc = bacc.Bacc(trn_type="TRN2", target_bir_lowering=False, debug=False)

    x16in = nc.dram_tensor("x16in", [NPC, C, L], f16, kind="ExternalInput")
    x8in = nc.dram_tensor("x8in", [NPC, C, L], f8, kind="ExternalInput")
    wt1e_d = nc.dram_tensor("wt1e", [C, len(C1_EXACT), C], f16, kind="ExternalInput")
    wt1q_d = nc.dram_tensor("wt1q", [C, len(C1_PAIRS), 2, C], f8, kind="ExternalInput")
    wt2e_d = nc.dram_tensor("wt2e", [C, len(C2_EXACT), C], f16, kind="ExternalInput")
    wt2q_d = nc.dram_tensor("wt2q", [C, len(C2_PAIRS), 2, C], f8, kind="ExternalInput")
    # columns: scale1, bias1, scale2, bias2
    vecs = nc.dram_tensor("vecs", [C, 4], f32, kind="ExternalInput")
    out = nc.dram_tensor("out", [NPC, C, H * W], f16, kind="ExternalOutput")

    def pair_ap(tile, base, delta):
        # overlapping [128, 2, NPIX] moving-operand AP for a DR tap pair
        return bass.AP(tile.tensor, tile.offset + base,
                       [[L, C], [delta, 2], [1, NPIX]])

    with TileContext(nc) as tc:
        with (
            tc.tile_pool(name="consts", bufs=1) as consts,
            tc.tile_pool(name="x16pool", bufs=2) as x16pool,
            tc.tile_pool(name="x8pool", bufs=2) as x8pool,
            tc.tile_pool(name="mid16pool", bufs=2) as mid16pool,
            tc.tile_pool(name="mid8pool", bufs=2) as mid8pool,
            tc.tile_pool(name="pspool", bufs=8, space="PSUM") as pspool,
            tc.tile_pool(name="otpool", bufs=2) as otpool,
            tc.tile_pool(name="stpool", bufs=3) as stpool,
        ):
            # First image's leading rows + conv1 weights go first so the PE
            # can start as early as possible.
            w1e_sb = consts.tile([C, len(C1_EXACT), C], f16, name="w1e_sb")
            w1q_sb = consts.tile([C, len(C1_PAIRS), 2, C], f8, name="w1q_sb")
            vecs_sb = consts.tile([C, 4], f32, name="vecs_sb")
            w2e_sb = consts.tile([C, len(C2_EXACT), C], f16, name="w2e_sb")
            w2q_sb = consts.tile([C, len(C2_PAIRS), 2, C], f8, name="w2q_sb")
            x0_16 = x16pool.tile([C, L], f16, name="x16", tag="x16")
            x0_8 = x8pool.tile([C, L], f8, name="x8", tag="x8")

            def img0_chunk(r0, r1):
                a, b = r0 * SW, (L if r1 >= LROWS else r1 * SW)
                nc.sync.dma_start(x0_16[:, a:b], x16in.ap()[0, :, a:b])
                nc.sync.dma_start(x0_8[:, a:b], x8in.ap()[0, :, a:b])

            img0_chunk(0, 8)
            nc.sync.dma_start(w1q_sb[:, :, :, :], wt1q_d.ap())
            nc.sync.dma_start(w1e_sb[:, :, :], wt1e_d.ap())
            img0_chunk(8, 24)
            nc.sync.dma_start(vecs_sb[:, :], vecs.ap())
            img0_chunk(24, 48)
            nc.sync.dma_start(w2q_sb[:, :, :, :], wt2q_d.ap())
            nc.sync.dma_start(w2e_sb[:, :, :], wt2e_d.ap())
            img0_chunk(48, 76)
            img0_chunk(76, LROWS)
            scale1 = vecs_sb[:, 0:1]
            bias1 = vecs_sb[:, 1:2]
            scale2 = vecs_sb[:, 2:3]
            bias2 = vecs_sb[:, 3:4]

            # Warm the PE HAM clock gate while the first DMAs are in flight
            # (cold PE runs at 1.2 GHz; ~3.4us of activity un-throttles it).
            warm_sb = consts.tile([C, 512], bf16, name="warm_sb")
            nc.vector.memset(warm_sb[:, :], 0.0)
            warm_ps = pspool.tile([C, 512], f32, name="warm_ps", tag="ps")
            for _ in range(8):
                nc.tensor.matmul(
                    warm_ps[:, :], warm_sb[:, 0:128], warm_sb[:, :],
                    start=True, stop=True,
                )

            for img in range(NPC):
                if img == 0:
                    x16 = x0_16
                    x8 = x0_8
                else:
                    x16 = x16pool.tile([C, L], f16, name="x16", tag="x16")
                    x8 = x8pool.tile([C, L], f8, name="x8", tag="x8")
                    for r0, r1 in ((0, 57 * SW), (57 * SW, L)):
                        nc.sync.dma_start(x16[:, r0:r1], x16in.ap()[img, :, r0:r1])
                        nc.sync.dma_start(x8[:, r0:r1], x8in.ap()[img, :, r0:r1])

                mid16 = mid16pool.tile([C, L], f16, name="mid16", tag="mid16")
                mid8 = mid8pool.tile([C, L], f8, name="mid8", tag="mid8")
                # zero borders: top pad row (incl row 0's separator), bottom
                # pad row + tail, interior separators (one per row)
                for m in (mid16, mid8):
                    m4 = m[:, 0 : SW * LROWS].rearrange("p (h w) -> p h w", h=LROWS)
                    nc.vector.memset(m[:, 0 : SW + 1], 0.0)
                    nc.vector.memset(m[:, SW * (LROWS - 1) : L], 0.0)
                    nc.vector.memset(m4[:, 2 : LROWS - 1, 0:1], 0.0)

                x4 = x16[:, 0 : SW * LROWS].rearrange("p (h w) -> p h w", h=LROWS)
                m16_4 = mid16[:, 0 : SW * LROWS].rearrange("p (h w) -> p h w", h=LROWS)
                m8_4 = mid8[:, 0 : SW * LROWS].rearrange("p (h w) -> p h w", h=LROWS)

                # ---- conv1 + bn1 + silu -> mid16 (fp16) and mid8 (fp8) ----
                for blk in range(NBLK):
                    h0 = blk * RB
                    base = SW * (h0 + 1) + 1
                    ps = pspool.tile([C, NPIX], f32, name="ps", tag="ps")
                    for p, (ta, tb) in enumerate(C1_PAIRS):
                        nc.tensor.matmul(
                            ps[:, :], w1q_sb[:, p, :, :],
                            pair_ap(x8, base + _toff(*ta), _toff(*tb) - _toff(*ta)),
                            start=(p == 0), stop=False, perf_mode=DR,
                        )
                    for j, t in enumerate(C1_EXACT):
                        o = base + _toff(*t)
                        nc.tensor.matmul(
                            ps[:, :], w1e_sb[:, j, :], x16[:, o : o + NPIX],
                            start=False, stop=(j == len(C1_EXACT) - 1),
                        )
                    ps3 = ps.rearrange("p (h w) -> p h w", h=RB)
                    nc.scalar.activation(
                        m16_4[:, h0 + 1 : h0 + 1 + RB, 1 : 1 + W],
                        ps3[:, :, 0:W],
                        Silu,
                        bias=bias1,
                        scale=scale1,
                    )
                    copy_eng = nc.gpsimd if blk % 3 == 0 else nc.vector
                    copy_eng.tensor_copy(
                        m8_4[:, h0 + 1 : h0 + 1 + RB, 1 : 1 + W],
                        m16_4[:, h0 + 1 : h0 + 1 + RB, 1 : 1 + W],
                    )

                # ---- conv2 + bn2 + residual + silu -> out ----
                GS = 4
                st = None
                for blk in range(NBLK):
                    h0 = blk * RB
                    base = SW * (h0 + 1) + 1
                    ps = pspool.tile([C, NPIX], f32, name="ps", tag="ps")
                    for p, (ta, tb) in enumerate(C2_PAIRS):
                        nc.tensor.matmul(
                            ps[:, :], w2q_sb[:, p, :, :],
                            pair_ap(mid8, base + _toff(*ta), _toff(*tb) - _toff(*ta)),
                            start=(p == 0), stop=False, perf_mode=DR,
                        )
                    for j, t in enumerate(C2_EXACT):
                        o = base + _toff(*t)
                        nc.tensor.matmul(
                            ps[:, :], w2e_sb[:, j, :], mid16[:, o : o + NPIX],
                            start=False, stop=(j == len(C2_EXACT) - 1),
                        )
                    ps3 = ps.rearrange("p (h w) -> p h w", h=RB)
                    xw = x4[:, h0 + 1 : h0 + 1 + RB, 1 : 1 + W]
                    last_group = img == NPC - 1 and blk >= NBLK - GS
                    if last_group:
                        # per-block epilogue+store at the very end shorten
                        # the tail chain after the final matmul
                        ot = otpool.tile([C, NOUT], f32, name="otl", tag="otl")
                        nc.vector.scalar_tensor_tensor(
                            ot.rearrange("p (h w) -> p h w", h=RB),
                            ps3[:, :, 0:W], scale2, xw, mult, add,
                        )
                        st = stpool.tile([C, GS * NOUT], f16, name="st", tag="st")
                        nc.scalar.activation(
                            st[:, 0:NOUT], ot[:, :], Silu, bias=bias2
                        )
                        nc.sync.dma_start(
                            out.ap()[img, :, h0 * W : (h0 + RB) * W], st[:, 0:NOUT]
                        )
                        continue
                    g = blk % GS
                    if g == 0:
                        ot = otpool.tile([C, GS * NOUT], f32, name="ot", tag="ot")
                        st = stpool.tile([C, GS * NOUT], f16, name="st", tag="st")
                    # fused: ot = ps*scale2 + x; silu bias folds into ACT
                    nc.vector.scalar_tensor_tensor(
                        ot[:, g * NOUT : (g + 1) * NOUT].rearrange(
                            "p (h w) -> p h w", h=RB
                        ),
                        ps3[:, :, 0:W], scale2, xw, mult, add,
                    )
                    if g == GS - 1:
                        nc.scalar.activation(st[:, :], ot[:, :], Silu, bias=bias2)
                        nc.sync.dma_start(
                            out.ap()[img, :, (h0 - (GS - 1) * RB) * W : (h0 + RB) * W],
                            st[:, :],
                        )

    nc.compile()
    return nc


def _quantize_ternary(w):
    """BitNet ternary quantization, matching the jax reference in fp32."""
    w = np.asarray(w, np.float32)
    scale = np.float32(max(np.float32(np.median(np.abs(w))), np.float32(1e-8)))
    tern = np.clip(np.round(w / scale), -1.0, 1.0).astype(np.float32)
    return tern, scale


def _pack_weights(tern, exact, pairs, f8dt):
    # lhsT layouts: [cin, tap, cout] fp16 and [cin, pair, 2, cout] fp8
    we = np.stack(
        [tern[:, :, dy + 1, dx + 1].T for (dy, dx) in exact], axis=1
    ).astype(np.float16)
    wq = np.stack(
        [
            np.stack([tern[:, :, ta[0] + 1, ta[1] + 1].T,
                      tern[:, :, tb[0] + 1, tb[1] + 1].T], axis=1)
            for (ta, tb) in pairs
        ],
        axis=1,
    ).astype(f8dt)
    return np.ascontiguousarray(we), np.ascontiguousarray(wq)


def _host_prep(x, w1, b1, g1, be1, m1, v1, w2, b2, g2, be2, m2, v2):
    t1, s1 = _quantize_ternary(w1)
    t2, s2 = _quantize_ternary(w2)
    f8 = ml_dtypes.float8_e4m3
    wt1e, wt1q = _pack_weights(t1, C1_EXACT, C1_PAIRS, f8)
    wt2e, wt2q = _pack_weights(t2, C2_EXACT, C2_PAIRS, f8)
    inv1 = (g1 / np.sqrt(v1 + BN_EPS)).astype(np.float32)
    inv2 = (g2 / np.sqrt(v2 + BN_EPS)).astype(np.float32)
    scale1 = s1 * inv1
    bias1 = b1 * inv1 + be1 - m1 * inv1
    scale2 = s2 * inv2
    bias2 = b2 * inv2 + be2 - m2 * inv2
    vecs = np.stack([scale1, bias1, scale2, bias2], axis=1).astype(np.float32)

    n = x.shape[0]
    x16 = np.zeros((n, C, L), dtype=np.float16)
    x8 = np.zeros((n, C, L), dtype=f8)
    for arr in (x16, x8):
        a4 = arr[:, :, 0 : SW * LROWS].reshape(n, C, LROWS, SW)
        a4[:, :, 1 : 1 + H, 1 : 1 + W] = x
    return x16, x8, wt1e, wt1q, wt2e, wt2q, vecs


def kernel(
    x,
    w1,
    b1,
    bn1_gamma,
    bn1_beta,
    bn1_mean,
    bn1_var,
    w2,
    b2,
    bn2_gamma,
    bn2_beta,
    bn2_mean,
    bn2_var,
    _trace=False,
):
    from concourse.bass_utils import run_bass_kernel_spmd

    x = np.asarray(x, np.float32)
    w1, b1, w2, b2 = (np.asarray(a, np.float32) for a in (w1, b1, w2, b2))
    bn1_gamma, bn1_beta, bn1_mean, bn1_var = (
        np.asarray(a, np.float32) for a in (bn1_gamma, bn1_beta, bn1_mean, bn1_var)
    )
    bn2_gamma, bn2_beta, bn2_mean, bn2_var = (
        np.asarray(a, np.float32) for a in (bn2_gamma, bn2_beta, bn2_mean, bn2_var)
    )

    x16, x8, wt1e, wt1q, wt2e, wt2q, vecs = _host_prep(
        x, w1, b1, bn1_gamma, bn1_beta, bn1_mean, bn1_var,
        w2, b2, bn2_gamma, bn2_beta, bn2_mean, bn2_var,
    )

    if "nc" not in _CACHE:
        _CACHE["nc"] = _build_nc()
    nc = _CACHE["nc"]

    in_maps = [
        {
            "x16in": np.ascontiguousarray(x16[i * NPC : (i + 1) * NPC]),
            "x8in": np.ascontiguousarray(x8[i * NPC : (i + 1) * NPC]),
            "wt1e": wt1e,
            "wt1q": wt1q,
            "wt2e": wt2e,
            "wt2q": wt2q,
            "vecs": vecs,
        }
        for i in range(NCORES)
    ]
    res = run_bass_kernel_spmd(nc, in_maps, core_ids=list(range(NCORES)), trace=_trace)
    outs = [
        res.results[i]["out"].reshape(NPC, C, H, W).astype(np.float32)
        for i in range(NCORES)
    ]
    full = np.concatenate(outs, axis=0)
    if _trace:
        _CACHE["last_results"] = res
    return full


# revision 10
# speedup vs baseline: 1.1020x; 1.1020x over previous
"""Trainium2 Bass kernel for nn_BasicBlockBit (ResNet BasicBlock, ternary convs).

Math (per reference):
    out = silu(bn2(conv3x3(silu(bn1(conv3x3(x, q(w1)) + b1)), q(w2)) + b2) + x)
with q() = BitNet ternary quantization (per-tensor median scale).

Strategy:
  - Pure data parallelism: batch 32 -> 4 images per core across 8 cores.
  - Mixed-precision taps: per 3x3 conv, a few taps run in fp16 (exact) and the
    rest run as fp8-e4m3 DoubleRow matmuls (2 taps per PE instruction at 2x
    ALU rate). conv1: 3 exact taps (middle row) + 3 DR pairs; conv2: 1 exact
    tap (center) + 4 DR pairs. Measured end-to-end rel err ~1.88e-2 < 2e-2.
  - Separator layout: image rows are stored with stride 113 (112 pixels + one
    zero column) plus a zero pad row above/below. Every 3x3 tap window of a
    4-row block is then one contiguous 452-element slab whose out-of-image
    reads land on zeros == exact zero padding. DR pairs use a custom
    overlapping AP [128, 2(step=tap delta), 452].
  - Epilogues: conv1: ACT Silu(ps*scale1+bias1) -> fp16 mid, DVE copy to fp8
    mid. conv2: DVE affine, DVE residual add (fp16 x), ACT Silu -> fp16
    staging -> DMA out (host upcasts to f32).
"""

import sys

import numpy as np
import ml_dtypes

try:  # concourse normally resolves via the environment's sitecustomize
    import concourse  # noqa: F401
except ImportError:  # pragma: no cover
    sys.path.insert(0, "/opt/trn_rl_repo")

C = 128
H = W = 112
SW = 113                 # row stride: 112 pixels + 1 zero separator
LROWS = 114              # pad row + 112 rows + pad row
L = SW * LROWS + 14      # 12896; window reads reach index 12883
NPC = 4                  # images per core
NCORES = 8
RB = 4                   # image rows per PSUM tile
NPIX = RB * SW           # 452 psum columns (448 valid)
NOUT = RB * W            # 448
NBLK = H // RB           # 28
BN_EPS = 1e-5

# taps as (dy, dx) in {-1,0,1}; offset in separator layout = 113*dy + dx
def _toff(dy, dx):
    return SW * dy + dx

# conv1: exact (fp16) taps = middle row; fp8 DR pairs (a first = smaller off)
C1_EXACT = [(0, -1), (0, 0), (0, 1)]
C1_PAIRS = [((-1, -1), (-1, 1)), ((-1, 0), (1, 0)), ((1, -1), (1, 1))]
# conv2: exact tap = center; 4 DR pairs
C2_EXACT = [(0, 0)]
C2_PAIRS = [((-1, -1), (-1, 1)), ((0, -1), (0, 1)), ((1, -1), (1, 1)),
            ((-1, 0), (1, 0))]

_CACHE = {}


def _build_nc():
    import concourse.mybir as mybir
    from concourse import bacc, bass
    from concourse.tile import TileContext

    f32 = mybir.dt.float32
    f16 = mybir.dt.float16
    f8 = mybir.dt.float8e4
    bf16 = mybir.dt.bfloat16
    DR = mybir.MatmulPerfMode.DoubleRow
    Silu = mybir.ActivationFunctionType.Silu
    mult = mybir.AluOpType.mult
    add = mybir.AluOpType.add

    nc = bacc.Bacc(trn_type="TRN2", target_bir_lowering=False, debug=False)

    x16in = nc.dram_tensor("x16in", [NPC, C, L], f16, kind="ExternalInput")
    x8in = nc.dram_tensor("x8in", [NPC, C, L], f8, kind="ExternalInput")
    wt1e_d = nc.dram_tensor("wt1e", [C, len(C1_EXACT), C], f16, kind="ExternalInput")
    wt1q_d = nc.dram_tensor("wt1q", [C, len(C1_PAIRS), 2, C], f8, kind="ExternalInput")
    wt2e_d = nc.dram_tensor("wt2e", [C, len(C2_EXACT), C], f16, kind="ExternalInput")
    wt2q_d = nc.dram_tensor("wt2q", [C, len(C2_PAIRS), 2, C], f8, kind="ExternalInput")
    # columns: scale1, bias1, scale2, bias2
    vecs = nc.dram_tensor("vecs", [C, 4], f32, kind="ExternalInput")
    out = nc.dram_tensor("out", [NPC, C, H * W], f16, kind="ExternalOutput")

    def pair_ap(tile, base, delta):
        # overlapping [128, 2, NPIX] moving-operand AP for a DR tap pair
        return bass.AP(tile.tensor, tile.offset + base,
                       [[L, C], [delta, 2], [1, NPIX]])

    with TileContext(nc) as tc:
        with (
            tc.tile_pool(name="consts", bufs=1) as consts,
            tc.tile_pool(name="x16pool", bufs=2) as x16pool,
            tc.tile_pool(name="x8pool", bufs=2) as x8pool,
            tc.tile_pool(name="mid16pool", bufs=2) as mid16pool,
            tc.tile_pool(name="mid8pool", bufs=2) as mid8pool,
            tc.tile_pool(name="pspool", bufs=8, space="PSUM") as pspool,
            tc.tile_pool(name="otpool", bufs=2) as otpool,
            tc.tile_pool(name="stpool", bufs=3) as stpool,
        ):
            # First image's leading rows + conv1 weights go first so the PE
            # can start as early as possible.
            w1e_sb = consts.tile([C, len(C1_EXACT), C], f16, name="w1e_sb")
            w1q_sb = consts.tile([C, len(C1_PAIRS), 2, C], f8, name="w1q_sb")
            vecs_sb = consts.tile([C, 4], f32, name="vecs_sb")
            w2e_sb = consts.tile([C, len(C2_EXACT), C], f16, name="w2e_sb")
            w2q_sb = consts.tile([C, len(C2_PAIRS), 2, C], f8, name="w2q_sb")
            x0_16 = x16pool.tile([C, L], f16, name="x16", tag="x16")
            x0_8 = x8pool.tile([C, L], f8, name="x8", tag="x8")

            def img0_chunk(r0, r1):
                a, b = r0 * SW, (L if r1 >= LROWS else r1 * SW)
                nc.sync.dma_start(x0_16[:, a:b], x16in.ap()[0, :, a:b])
                nc.sync.dma_start(x0_8[:, a:b], x8in.ap()[0, :, a:b])

            img0_chunk(0, 8)
            nc.sync.dma_start(w1q_sb[:, :, :, :], wt1q_d.ap())
            nc.sync.dma_start(w1e_sb[:, :, :], wt1e_d.ap())
            img0_chunk(8, 24)
            nc.sync.dma_start(vecs_sb[:, :], vecs.ap())
            img0_chunk(24, 48)
            nc.sync.dma_start(w2q_sb[:, :, :, :], wt2q_d.ap())
            nc.sync.dma_start(w2e_sb[:, :, :], wt2e_d.ap())
            img0_chunk(48, 76)
            img0_chunk(76, LROWS)
            scale1 = vecs_sb[:, 0:1]
            bias1 = vecs_sb[:, 1:2]
            scale2 = vecs_sb[:, 2:3]
            bias2 = vecs_sb[:, 3:4]

            # Warm the PE HAM clock gate while the first DMAs are in flight
            # (cold PE runs at 1.2 GHz; ~3.4us of activity un-throttles it).
            warm_sb = consts.tile([C, 512], bf16, name="warm_sb")
            nc.vector.memset(warm_sb[:, :], 0.0)
            warm_ps = pspool.tile([C, 512], f32, name="warm_ps", tag="ps")
            for _ in range(8):
                nc.tensor.matmul(
                    warm_ps[:, :], warm_sb[:, 0:128], warm_sb[:, :],
                    start=True, stop=True,
                )

            for img in range(NPC):
                if img == 0:
                    x16 = x0_16
                    x8 = x0_8
                else:
                    x16 = x16pool.tile([C, L], f16, name="x16", tag="x16")
                    x8 = x8pool.tile([C, L], f8, name="x8", tag="x8")
                    for r0, r1 in ((0, 57 * SW), (57 * SW, L)):
                        nc.sync.dma_start(x16[:, r0:r1], x16in.ap()[img, :, r0:r1])
                        nc.sync.dma_start(x8[:, r0:r1], x8in.ap()[img, :, r0:r1])

                mid16 = mid16pool.tile([C, L], f16, name="mid16", tag="mid16")
                mid8 = mid8pool.tile([C, L], f8, name="mid8", tag="mid8")
                # zero borders: top pad row (incl row 0's separator), bottom
                # pad row + tail, interior separators (one per row)
                for m in (mid16, mid8):
                    m4 = m[:, 0 : SW * LROWS].rearrange("p (h w) -> p h w", h=LROWS)
                    nc.vector.memset(m[:, 0 : SW + 1], 0.0)
                    nc.vector.memset(m[:, SW * (LROWS - 1) : L], 0.0)
                    nc.vector.memset(m4[:, 2 : LROWS - 1, 0:1], 0.0)

                x4 = x16[:, 0 : SW * LROWS].rearrange("p (h w) -> p h w", h=LROWS)
                m16_4 = mid16[:, 0 : SW * LROWS].rearrange("p (h w) -> p h w", h=LROWS)
                m8_4 = mid8[:, 0 : SW * LROWS].rearrange("p (h w) -> p h w", h=LROWS)

                # ---- conv1 + bn1 + silu -> mid16 (fp16) and mid8 (fp8) ----
                for blk in range(NBLK):
                    h0 = blk * RB
                    base = SW * (h0 + 1) + 1
                    ps = pspool.tile([C, NPIX], f32, name="ps", tag="ps")
                    for p, (ta, tb) in enumerate(C1_PAIRS):
                        nc.tensor.matmul(
                            ps[:, :], w1q_sb[:, p, :, :],
                            pair_ap(x8, base + _toff(*ta), _toff(*tb) - _toff(*ta)),
                            start=(p == 0), stop=False, perf_mode=DR,
                        )
                    for j, t in enumerate(C1_EXACT):
                        o = base + _toff(*t)
                        nc.tensor.matmul(
                            ps[:, :], w1e_sb[:, j, :], x16[:, o : o + NPIX],
                            start=False, stop=(j == len(C1_EXACT) - 1),
                        )
                    ps3 = ps.rearrange("p (h w) -> p h w", h=RB)
                    nc.scalar.activation(
                        m16_4[:, h0 + 1 : h0 + 1 + RB, 1 : 1 + W],
                        ps3[:, :, 0:W],
                        Silu,
                        bias=bias1,
                        scale=scale1,
                    )
                    # contiguous 452-span copy (separators are zero in both)
                    nc.gpsimd.tensor_copy(
                        mid8[:, base - 1 : base - 1 + NPIX],
                        mid16[:, base - 1 : base - 1 + NPIX],
                    )

                # ---- conv2 + bn2 + residual + silu -> out ----
                GS = 4
                st = None
                for blk in range(NBLK):
                    h0 = blk * RB
                    base = SW * (h0 + 1) + 1
                    ps = pspool.tile([C, NPIX], f32, name="ps", tag="ps")
                    for p, (ta, tb) in enumerate(C2_PAIRS):
                        nc.tensor.matmul(
                            ps[:, :], w2q_sb[:, p, :, :],
                            pair_ap(mid8, base + _toff(*ta), _toff(*tb) - _toff(*ta)),
                            start=(p == 0), stop=False, perf_mode=DR,
                        )
                    for j, t in enumerate(C2_EXACT):
                        o = base + _toff(*t)
                        nc.tensor.matmul(
                            ps[:, :], w2e_sb[:, j, :], mid16[:, o : o + NPIX],
                            start=False, stop=(j == len(C2_EXACT) - 1),
                        )
                    # all-contiguous epilogue: garbage separator columns flow
                    # through stt+ACT; the output DMA skips them
                    xw = x16[:, base : base + NPIX]
                    last_group = img == NPC - 1 and blk >= NBLK - GS
                    if last_group:
                        # per-block epilogue+store at the very end shorten
                        # the tail chain after the final matmul
                        ot = otpool.tile([C, NPIX], f32, name="otl", tag="otl")
                        nc.vector.scalar_tensor_tensor(
                            ot[:, :], ps[:, :], scale2, xw, mult, add
                        )
                        st = stpool.tile([C, GS * NPIX], f16, name="st", tag="st")
                        nc.scalar.activation(
                            st[:, 0:NPIX], ot[:, :], Silu, bias=bias2
                        )
                        nc.sync.dma_start(
                            out.ap()[img, :, h0 * W : (h0 + RB) * W],
                            st[:, 0:NPIX].rearrange("p (h w) -> p h w", w=SW)[
                                :, :, 0:W
                            ],
                        )
                        continue
                    g = blk % GS
                    if g == 0:
                        ot = otpool.tile([C, GS * NPIX], f32, name="ot", tag="ot")
                        st = stpool.tile([C, GS * NPIX], f16, name="st", tag="st")
                    # fused: ot = ps*scale2 + x; silu bias folds into ACT
                    nc.vector.scalar_tensor_tensor(
                        ot[:, g * NPIX : (g + 1) * NPIX], ps[:, :], scale2, xw,
                        mult, add,
                    )
                    if g == GS - 1:
                        nc.scalar.activation(st[:, :], ot[:, :], Silu, bias=bias2)
                        nc.sync.dma_start(
                            out.ap()[img, :, (h0 - (GS - 1) * RB) * W : (h0 + RB) * W],
                            st.rearrange("p (h w) -> p h w", w=SW)[:, :, 0:W],
                        )

    nc.compile()
    return nc


def _quantize_ternary(w):
    """BitNet ternary quantization, matching the jax reference in fp32."""
    w = np.asarray(w, np.float32)
    scale = np.float32(max(np.float32(np.median(np.abs(w))), np.float32(1e-8)))
    tern = np.clip(np.round(w / scale), -1.0, 1.0).astype(np.float32)
    return tern, scale


def _pack_weights(tern, exact, pairs, f8dt):
    # lhsT layouts: [cin, tap, cout] fp16 and [cin, pair, 2, cout] fp8
    we = np.stack(
        [tern[:, :, dy + 1, dx + 1].T for (dy, dx) in exact], axis=1
    ).astype(np.float16)
    wq = np.stack(
        [
            np.stack([tern[:, :, ta[0] + 1, ta[1] + 1].T,
                      tern[:, :, tb[0] + 1, tb[1] + 1].T], axis=1)
            for (ta, tb) in pairs
        ],
        axis=1,
    ).astype(f8dt)
    return np.ascontiguousarray(we), np.ascontiguousarray(wq)


def _host_prep(x, w1, b1, g1, be1, m1, v1, w2, b2, g2, be2, m2, v2):
    t1, s1 = _quantize_ternary(w1)
    t2, s2 = _quantize_ternary(w2)
    f8 = ml_dtypes.float8_e4m3
    wt1e, wt1q = _pack_weights(t1, C1_EXACT, C1_PAIRS, f8)
    wt2e, wt2q = _pack_weights(t2, C2_EXACT, C2_PAIRS, f8)
    inv1 = (g1 / np.sqrt(v1 + BN_EPS)).astype(np.float32)
    inv2 = (g2 / np.sqrt(v2 + BN_EPS)).astype(np.float32)
    scale1 = s1 * inv1
    bias1 = b1 * inv1 + be1 - m1 * inv1
    scale2 = s2 * inv2
    bias2 = b2 * inv2 + be2 - m2 * inv2
    vecs = np.stack([scale1, bias1, scale2, bias2], axis=1).astype(np.float32)

    n = x.shape[0]
    x16 = np.zeros((n, C, L), dtype=np.float16)
    x8 = np.zeros((n, C, L), dtype=f8)
    for arr in (x16, x8):
        a4 = arr[:, :, 0 : SW * LROWS].reshape(n, C, LROWS, SW)
        a4[:, :, 1 : 1 + H, 1 : 1 + W] = x
    return x16, x8, wt1e, wt1q, wt2e, wt2q, vecs


def kernel(
    x,
    w1,
    b1,
    bn1_gamma,
    bn1_beta,
    bn1_mean,
    bn1_var,
    w2,
    b2,
    bn2_gamma,
    bn2_beta,
    bn2_mean,
    bn2_var,
    _trace=False,
):
    from concourse.bass_utils import run_bass_kernel_spmd

    x = np.asarray(x, np.float32)
    w1, b1, w2, b2 = (np.asarray(a, np.float32) for a in (w1, b1, w2, b2))
    bn1_gamma, bn1_beta, bn1_mean, bn1_var = (
        np.asarray(a, np.float32) for a in (bn1_gamma, bn1_beta, bn1_mean, bn1_var)
    )
    bn2_gamma, bn2_beta, bn2_mean, bn2_var = (
        np.asarray(a, np.float32) for a in (bn2_gamma, bn2_beta, bn2_mean, bn2_var)
    )

    x16, x8, wt1e, wt1q, wt2e, wt2q, vecs = _host_prep(
        x, w1, b1, bn1_gamma, bn1_beta, bn1_mean, bn1_var,
        w2, b2, bn2_gamma, bn2_beta, bn2_mean, bn2_var,
    )

    if "nc" not in _CACHE:
        _CACHE["nc"] = _build_nc()
    nc = _CACHE["nc"]

    in_maps = [
        {
            "x16in": np.ascontiguousarray(x16[i * NPC : (i + 1) * NPC]),
            "x8in": np.ascontiguousarray(x8[i * NPC : (i + 1) * NPC]),
            "wt1e": wt1e,
            "wt1q": wt1q,
            "wt2e": wt2e,
            "wt2q": wt2q,
            "vecs": vecs,
        }
        for i in range(NCORES)
    ]
    res = run_bass_kernel_spmd(nc, in_maps, core_ids=list(range(NCORES)), trace=_trace)
    outs = [
        res.results[i]["out"].reshape(NPC, C, H, W).astype(np.float32)
        for i in range(NCORES)
    ]
    full = np.concatenate(outs, axis=0)
    if _trace:
        _CACHE["last_results"] = res
    return full
